# revision 17
# baseline (speedup 1.0000x reference)
"""LSH similarity-matrix kernel for Trainium2 (8 NeuronCores, data-parallel over batch).

Math: reference computes, per (l, b):
    c1 = (query_embed @ r.T > 0),  c2 = (doc_embed @ r.T > 0)   in {0,1}
    ham = s1 + s2 - 2*c1@c2.T ;  sim = cos(pi/NB * ham), masked where tok==0.
With +-1 codes U = 2c-1 and S = U1 @ U2.T:  ham = (NB - S)/2, so
    sim = sin(pi/(2*NB) * S).
Masks fold into the embeddings: a zeroed embedding row projects to 0,
clamp(0) = 0 gives a zero code row, so S = 0 and sin(0) = 0 — exactly the
masked output. Masked doc tokens (half of them: tok in {0,1}) are gathered
away host-side entirely; output columns scatter back as zeros. Batches are
assigned to (core, slot) sorted by active-token count so every core runs an
identically-shaped program with minimal padding per slot.

Precision: projections run as single tf32 (float32r) matmuls at 1 cycle/row.
tf32's 11-bit mantissa flips ~1.5k of the 71M hash bits vs the fp32
reference (sim absmax ~9e-3, rel err ~1e-4) — far inside the tolerance.
Inputs are pre-rounded to tf32 host-side and DMA'd straight into float32r
tiles, so no on-device conversion copies are needed. The code dot runs as
fp8e4m3 DoubleRow matmuls (chunk pairs give K=256 per MM at 2 MACs/cell/
cycle); +-1/0 codes and their fp32 PSUM accumulation are exact.

r is pre-scaled by 2^66 host-side so the sign alternative
clamp(x, -1, 1) = max(min(x,1),-1) is exact (any |proj| > 2^-66 maps to
+-1). The PSUM->SBUF sign drain is the second bottleneck after the PE
(GPSIMD cannot read PSUM, so only DVE and ACT can do it); chunk signs are
assigned to DVE (clamp tensor_scalar) or ACT (Sign activation) by a greedy
cost balancer that pre-charges ACT with the per-job Sin epilogue and its
act-table loads. Output is written bf16 (exact enough; halves output DMA)
and upcast on the host.
"""
import os
import sys

sys.path.insert(0, "/opt/trn_rl_repo")

from contextlib import ExitStack

import numpy as np

import concourse.bass as bass
import concourse.mybir as mybir
import concourse.tile as tile
from concourse import bacc
from concourse.bass_utils import run_bass_kernel_spmd

L, BAT, A, BDOC, D, NB = 2, 32, 64, 1024, 128, 1024
CORES = 8
BPC = BAT // CORES          # batch slots per core
CH = NB // 128              # 8 bit-chunks
SCALE = float(2.0 ** 66)
PI = float(np.pi)

F32 = mybir.dt.float32
F32R = mybir.dt.float32r
BF16 = mybir.dt.bfloat16
FP8 = mybir.dt.float8e4
Alu = mybir.AluOpType
Act = mybir.ActivationFunctionType

NWARM = 6                   # PE ramp dummy matmuls (512 cols each)

_BUILD_CACHE: dict = {}


def _col_splits(n):
    """Split [0, n) into equal-width pieces of <=512 columns (>=256 keeps
    float32r matmuls at full rate; a matmul may not cross a PSUM bank, so
    piece i is written at PSUM column 512*i). Equal widths mean one strided
    [p, npieces, w] access pattern covers all pieces, so sign/sin run as a
    single instruction per chunk. Returns (c0, c1, p0) per piece."""
    npieces = -(-n // 512)
    w = -(-(n // npieces) // 16) * 16
    while w * npieces < n:
        w += 16
    assert w * npieces >= n and w <= 512 and npieces <= 2
    return [(i * w, min((i + 1) * w, n), 512 * i) for i in range(npieces)]


def _sign_plan(pads_c, qpad):
    """Assign each (job, chunk) doc sign to 'dve' or 'act' greedily by
    modelled engine cost (ns): DVE tensor_scalar = free*1.042 + 125,
    ACT activation = free*0.833 + 143. Costs are charged at the point in
    the job stream where the work actually runs (sin of job j-1 lands
    during job j; query-pair signs land around job 1; act-table loads are
    hoisted to the idle warm-up window) so the split is balanced in TIME,
    not just in total. Jobs run slots in descending-pad order, L-major."""
    QW = BPC * L * qpad
    order = sorted(range(BPC), key=lambda s: -pads_c[s])
    jobs = [(b, l) for b in order for l in range(L)]
    dve = 0.0
    act = 0.0
    plan = []
    for j, (b, _l) in enumerate(jobs):
        if j == 1:
            dve += 2 * (2 * QW * 1.042 + 125.0)
            act += 2 * (2 * QW * 0.833 + 212.0)
        if j >= 2:
            act += pads_c[jobs[j - 2][0]] * 0.833 + 212.0
        row = []
        for _k in range(CH):
            cd = pads_c[b] * 1.042 + 125.0
            ca = pads_c[b] * 0.833 + 212.0
            if dve + cd <= act + ca:
                dve += cd
                row.append("dve")
            else:
                act += ca
                row.append("act")
        plan.append(row)
    return plan


def _build(pads_c: tuple, qpad: int = A, reps: int = 1):
    """Per-core SPMD program. pads_c[b]: compute width (mult of 16) of batch
    slot b. reps > 1 re-emits the whole body (timing instrumentation only)."""
    pads_c = tuple(int(p) for p in pads_c)
    pad_cmax = max(pads_c)
    slot_splits = [_col_splits(p) for p in pads_c]
    QW = BPC * L * qpad
    sign_plan = _sign_plan(pads_c, qpad)

    nc = bacc.Bacc("TRN2", target_bir_lowering=False, debug=False)

    QE = nc.dram_tensor("qe", [D, QW], F32R, kind="ExternalInput").ap()
    DE = nc.dram_tensor("de", [BPC, L, D, pad_cmax], F32R, kind="ExternalInput").ap()
    RT = nc.dram_tensor("rt", [D, NB], F32R, kind="ExternalInput").ap()
    OUT = nc.dram_tensor("out", [BPC, L, qpad, pad_cmax], BF16, kind="ExternalOutput").ap()

    def sign_to(eng, u, pp):
        if eng == "dve":
            nc.vector.tensor_scalar(u, pp, 1.0, -1.0, Alu.min, Alu.max)
        else:
            nc.scalar.activation(u, pp, Act.Sign)

    with tile.TileContext(nc) as tc, ExitStack() as ctx:
        const = ctx.enter_context(tc.tile_pool(name="const", bufs=1))
        ehp = ctx.enter_context(tc.tile_pool(name="ehp", bufs=4))
        u2p = ctx.enter_context(tc.tile_pool(name="u2p", bufs=3))
        outp = ctx.enter_context(tc.tile_pool(name="outp", bufs=2))
        # 8 PSUM banks: 2 x 2-bank rotating chunk tiles + 2 x 2-bank S tiles
        # (two S bufs so job j's dots never wait on job j-1's Sin drain)
        ps_p = ctx.enter_context(tc.tile_pool(name="ps_p", bufs=2, space="PSUM"))
        ps_s = ctx.enter_context(tc.tile_pool(name="ps_s", bufs=2, space="PSUM"))

        for _rep in range(reps):
            _rp = f"r{_rep}_"
            # ---- constants: rt arrives in pieces so the first projection
            # chunk unblocks as early as possible; everything lands directly
            # in float32r tiles (host pre-rounds to tf32) ----
            rhl = const.tile([D, NB], F32R, tag="rhl", name=f"{_rp}rhl")
            qh = const.tile([D, QW], F32R, tag="qh", name=f"{_rp}qh")
            U1 = const.tile([D, CH * QW], FP8, tag="U1", name=f"{_rp}U1")

            _slot_order = sorted(range(BPC), key=lambda s: -pads_c[s])
            jobs = [(b, l) for b in _slot_order for l in range(L)]
            st = [dict() for _ in jobs]

            def stage_a(j):
                b, l = jobs[j]
                pad = pads_c[b]
                eh = ehp.tile([D, pad_cmax], F32R, tag="eh",
                              name=f"{_rp}eh{j}")[:, 0:pad]
                nc.sync.dma_start(out=eh, in_=DE[b, l, :, 0:pad])
                st[j]["eh"] = eh

            # DMA priority order: first doc job, first proj chunk weights,
            # the rest of the weights, second doc job, queries.
            stage_a(0)
            nc.sync.dma_start(out=rhl[:, 0:128], in_=RT[:, 0:128])
            nc.sync.dma_start(out=rhl[:, 128:NB], in_=RT[:, 128:NB])
            nc.sync.dma_start(out=qh, in_=QE)
            stage_a(1)

            # PE pre-warm: dependency-free dummy matmuls pull the PE through
            # its cold/mid clock ramp while the first DMAs land, so the real
            # projections run at 2.4 GHz
            warm = const.tile([D, 512], BF16, tag="warm", name=f"{_rp}warm")
            nc.gpsimd.memset(warm, 0.0)
            # dummy Sign + Sin on the idle ACT engine so both act-table
            # loads are hoisted into the warm-up window instead of stalling
            # the pipeline at their first real use
            wact = const.tile([D, 32], BF16, tag="wact", name=f"{_rp}wact")
            nc.scalar.activation(wact[:, 0:16], warm[:, 0:16], Act.Sign)
            nc.scalar.activation(wact[:, 16:32], warm[:, 16:32], Act.Sin)
            wps = ps_p.tile([D, 1024], F32, tag="pp",
                            name=f"{_rp}wps")[:, 0:512]
            for i in range(NWARM):
                nc.tensor.matmul(wps, warm[:, 0:128], warm,
                                 start=True, stop=True)

            def stage_b(j):
                b, l = jobs[j]
                pad = pads_c[b]
                splits = slot_splits[b]
                npieces = len(splits)
                w = splits[0][1] - splits[0][0]
                assert npieces * w == pad
                eh = st[j]["eh"]
                U2 = u2p.tile([D, CH * pad_cmax], FP8, tag="U2",
                              name=f"{_rp}U2{j}")
                for k in range(CH):
                    rh_k = rhl[:, k * 128:(k + 1) * 128]
                    pp = ps_p.tile([D, 1024], F32, tag="pp",
                                   name=f"{_rp}pp{j}_{k}")
                    for c0, c1, p0 in splits:
                        nc.tensor.matmul(pp[:, p0:p0 + c1 - c0], rh_k,
                                         eh[:, c0:c1], start=True, stop=True)
                    if npieces == 1:
                        ppv = pp[:, 0:pad]
                        u2v = U2[:, k * pad:(k + 1) * pad]
                    else:
                        ppv = pp[:].rearrange("p (n c) -> p n c",
                                              c=512)[:, 0:npieces, 0:w]
                        u2v = U2[:, k * pad:(k + 1) * pad] \
                            .rearrange("p (n c) -> p n c", c=w)
                    sign_to(sign_plan[j][k], u2v, ppv)
                st[j]["U2"] = U2

            def query_proj():
                # chunk pairs share one PSUM tile (cols 0 and 512) so the
                # sign runs as one fused op per pair, alternating DVE/ACT
                # so all four signs finish before the first dot needs U1
                for kk in range(CH // 2):
                    qp = ps_p.tile([D, 1024], F32, tag="pp",
                                   name=f"{_rp}qp{kk}")
                    for h in range(2):
                        k = 2 * kk + h
                        nc.tensor.matmul(qp[:, 512 * h:512 * h + QW],
                                         rhl[:, k * 128:(k + 1) * 128], qh,
                                         start=True, stop=True)
                    u1v = U1[:, 2 * kk * QW:(2 * kk + 2) * QW] \
                        .rearrange("p (n c) -> p n c", c=QW)
                    qpv = qp[:].rearrange("p (n c) -> p n c",
                                          c=512)[:, 0:2, 0:QW]
                    sign_to("dve" if kk % 2 == 0 else "act", u1v, qpv)

            def stage_c(j):
                b, l = jobs[j]
                pad = pads_c[b]
                splits = slot_splits[b]
                npieces = len(splits)
                w = splits[0][1] - splits[0][0]
                U2 = st[j]["U2"]
                # code dot via fp8 DoubleRow: chunk pairs (2jj, 2jj+1) fold
                # into one K=256 matmul; +-1/0 codes are exact in fp8e4m3
                S = ps_s.tile([qpad, 1024], F32, tag="S",
                              name=f"{_rp}S{j}")
                qcol = (b * L + l) * qpad
                for c0, c1, p0 in splits:
                    for jj in range(CH // 2):
                        lw = U1[:, 2 * jj * QW:(2 * jj + 2) * QW] \
                            .rearrange("p (o c) -> p o c", o=2)[:, :, qcol:qcol + qpad]
                        rv = U2[:, 2 * jj * pad:(2 * jj + 2) * pad] \
                            .rearrange("p (o c) -> p o c", o=2)[:, :, c0:c1]
                        nc.tensor.matmul(
                            S[:, p0:p0 + c1 - c0], lw, rv,
                            start=(jj == 0), stop=(jj == CH // 2 - 1),
                            perf_mode=mybir.MatmulPerfMode.DoubleRow,
                        )
                sim = outp.tile([qpad, pad_cmax], BF16, tag="sim",
                                name=f"{_rp}sim{j}")[:, 0:pad]
                if npieces == 1:
                    nc.scalar.activation(sim, S[:, 0:pad], Act.Sin,
                                         scale=PI / (2.0 * NB))
                else:
                    sv = S[:].rearrange("p (n c) -> p n c",
                                        c=512)[:, 0:npieces, 0:w]
                    mv = sim.rearrange("p (n c) -> p n c", c=w)
                    nc.scalar.activation(mv, sv, Act.Sin, scale=PI / (2.0 * NB))
                nc.sync.dma_start(out=OUT[b, l, :, 0:pad], in_=sim)

            # deeper pipeline: c(j) trails b(j+1), so dots/sin/output of job
            # j overlap the projections of job j+2 and the PE never waits
            # on the sign engines at job boundaries
            n = len(jobs)
            stage_b(0)
            stage_a(2)
            query_proj()
            stage_b(1)
            stage_a(3)
            for j in range(n):
                stage_c(j)
                if j + 4 < n:
                    stage_a(j + 4)
                if j + 2 < n:
                    stage_b(j + 2)

    nc.compile()
    return nc


def _tf32(x):
    """Round-to-nearest-even fp32 -> tf32 (11-bit mantissa), bit-matching
    the PE's fp32_to_fp32r conversion."""
    u = np.ascontiguousarray(x, np.float32).view(np.uint32).astype(np.uint64)
    u = (u + 0x07FF + ((u >> 12) & 1)) & 0xFFFFFFFFFFFFF000
    return (u & 0xFFFFFFFF).astype(np.uint32).view(np.float32)


def _stage_inputs(query_embed, doc_embed, query_tok, doc_tok, r):
    query_embed = np.ascontiguousarray(query_embed, dtype=np.float32)
    doc_embed = np.ascontiguousarray(doc_embed, dtype=np.float32)
    r = np.ascontiguousarray(r, dtype=np.float32)

    qmask = (np.asarray(query_tok) != 0)
    dmask = (np.asarray(doc_tok) != 0)

    # sort batches by active count; slot s takes ranks [s*CORES, (s+1)*CORES)
    # spread across the 8 cores, so per-slot padding is tight and identical
    # on every core (SPMD requires one shape per slot)
    counts = dmask.sum(axis=1).astype(int)
    order = np.argsort(counts, kind="stable")
    assign = np.empty((CORES, BPC), dtype=int)   # assign[c, b] = batch id
    for s in range(BPC):
        for c in range(CORES):
            assign[c, s] = order[s * CORES + c]
    def _pad(n):
        # mult of 16; slots that split across two PSUM banks need halves
        # that are themselves mult of 16, so round those to mult of 32
        p = max(64, -(-n // 16) * 16)
        if p > 512:
            p = -(-n // 32) * 32
        return min(BDOC, p)

    pads_c = tuple(_pad(int(counts[assign[:, s]].max())) for s in range(BPC))
    pad_cmax = max(pads_c)

    qe_m = query_embed * qmask[None, :, :, None].astype(np.float32)
    qidxs = [np.flatnonzero(qmask[g]) for g in range(BAT)]
    qpad = min(A, max(16, int(-(-max(len(q) for q in qidxs) // 16) * 16)))
    rt = np.ascontiguousarray(_tf32(r.T * SCALE))

    idxs = [np.flatnonzero(dmask[g]) for g in range(BAT)]
    in_maps = []
    for c in range(CORES):
        # embeddings staged pre-transposed [D, tokens], pre-rounded to tf32
        # (value-exact under the f32r DMA interpretation); queries compacted
        # to their active rows (masks are per-batch, shared by both layers)
        qe_c = np.zeros((D, BPC * L * qpad), dtype=np.float32)
        de_c = np.zeros((BPC, L, D, pad_cmax), dtype=np.float32)
        for b in range(BPC):
            g = assign[c, b]
            qi = qidxs[g]
            for li in range(L):
                col = (b * L + li) * qpad
                qe_c[:, col:col + len(qi)] = qe_m[li, g, qi].T
            idx = idxs[g]
            de_c[b, :, :, :len(idx)] = doc_embed[:, g, idx].transpose(0, 2, 1)
        in_maps.append({"qe": _tf32(qe_c), "de": _tf32(de_c), "rt": rt})

    return in_maps, assign, idxs, pads_c, qidxs, qpad


def kernel(query_embed, doc_embed, query_tok, doc_tok, r):
    in_maps, assign, idxs, pads_c, qidxs, qpad = _stage_inputs(
        query_embed, doc_embed, query_tok, doc_tok, r)

    key = (pads_c, qpad)
    if key not in _BUILD_CACHE:
        _BUILD_CACHE[key] = _build(pads_c, qpad)
    nc = _BUILD_CACHE[key]

    res = run_bass_kernel_spmd(nc, in_maps, core_ids=list(range(CORES)))

    out = np.zeros((BAT, L, A, BDOC), dtype=np.float32)
    for c in range(CORES):
        o_c = np.asarray(res.results[c]["out"]).astype(np.float32)
        for b in range(BPC):
            g = assign[c, b]
            idx = idxs[g]
            qi = qidxs[g]
            for li in range(L):
                out[g, li][np.ix_(qi, idx)] = o_c[b, li, :len(qi), :len(idx)]
    return out


# revision 18
# speedup vs baseline: 1.2467x; 1.2467x over previous
"""LSH similarity-matrix kernel for Trainium2 (8 NeuronCores, data-parallel over batch).

Math: reference computes, per (l, b):
    c1 = (query_embed @ r.T > 0),  c2 = (doc_embed @ r.T > 0)   in {0,1}
    ham = s1 + s2 - 2*c1@c2.T ;  sim = cos(pi/NB * ham), masked where tok==0.
With +-1 codes U = 2c-1 and S = U1 @ U2.T:  ham = (NB - S)/2, so
    sim = sin(pi/(2*NB) * S).
Masks fold into the embeddings: a zeroed embedding row projects to 0,
clamp(0) = 0 gives a zero code row, so S = 0 and sin(0) = 0 — exactly the
masked output. Masked doc tokens (half of them: tok in {0,1}) are gathered
away host-side entirely; output columns scatter back as zeros. Batches are
assigned to (core, slot) sorted by active-token count so every core runs an
identically-shaped program with minimal padding per slot.

Precision: projections run as single tf32 (float32r) matmuls at 1 cycle/row.
tf32's 11-bit mantissa flips ~1.5k of the 71M hash bits vs the fp32
reference (sim absmax ~9e-3, rel err ~1e-4) — far inside the tolerance.
Inputs are pre-rounded to tf32 host-side and DMA'd straight into float32r
tiles, so no on-device conversion copies are needed. The code dot runs as
fp8e4m3 DoubleRow matmuls (chunk pairs give K=256 per MM at 2 MACs/cell/
cycle); +-1/0 codes and their fp32 PSUM accumulation are exact.

r is pre-scaled by 2^66 host-side so the sign alternative
clamp(x, -1, 1) = max(min(x,1),-1) is exact (any |proj| > 2^-66 maps to
+-1). The PSUM->SBUF sign drain is the second bottleneck after the PE
(GPSIMD cannot read PSUM, so only DVE and ACT can do it); chunk signs are
assigned to DVE (clamp tensor_scalar) or ACT (Sign activation) by a greedy
cost balancer that pre-charges ACT with the per-job Sin epilogue and its
act-table loads. Output is written bf16 (exact enough; halves output DMA)
and upcast on the host.
"""
import os
import sys

sys.path.insert(0, "/opt/trn_rl_repo")

from contextlib import ExitStack

import numpy as np

import concourse.bass as bass
import concourse.mybir as mybir
import concourse.tile as tile
from concourse import bacc
from concourse.bass_utils import run_bass_kernel_spmd

L, BAT, A, BDOC, D, NB = 2, 32, 64, 1024, 128, 1024
CORES = 8
BPC = BAT // CORES          # batch slots per core
CH = NB // 128              # 8 bit-chunks
SCALE = float(2.0 ** 66)
PI = float(np.pi)

F32 = mybir.dt.float32
F32R = mybir.dt.float32r
BF16 = mybir.dt.bfloat16
FP8 = mybir.dt.float8e4
Alu = mybir.AluOpType
Act = mybir.ActivationFunctionType

NWARM = 6                   # PE ramp dummy matmuls (512 cols each)

_BUILD_CACHE: dict = {}


def _col_splits(n):
    """Split [0, n) into equal-width pieces of <=512 columns (>=256 keeps
    float32r matmuls at full rate; a matmul may not cross a PSUM bank, so
    piece i is written at PSUM column 512*i). Equal widths mean one strided
    [p, npieces, w] access pattern covers all pieces, so sign/sin run as a
    single instruction per chunk. Returns (c0, c1, p0) per piece."""
    npieces = -(-n // 512)
    w = -(-(n // npieces) // 16) * 16
    while w * npieces < n:
        w += 16
    assert w * npieces >= n and w <= 512 and npieces <= 2
    return [(i * w, min((i + 1) * w, n), 512 * i) for i in range(npieces)]


def _sign_plan(pads_c, qpad):
    """Assign each (job, chunk) doc sign to 'dve' or 'act' greedily by
    modelled engine cost (ns): DVE tensor_scalar = free*1.042 + 125,
    ACT activation = free*0.833 + 143. Costs are charged at the point in
    the job stream where the work actually runs (sin of job j-1 lands
    during job j; query-pair signs land around job 1; act-table loads are
    hoisted to the idle warm-up window) so the split is balanced in TIME,
    not just in total. Jobs run slots in descending-pad order, L-major."""
    QW = BPC * L * qpad
    order = sorted(range(BPC), key=lambda s: -pads_c[s])
    jobs = [(b, l) for b in order for l in range(L)]
    dve = 0.0
    act = 0.0
    plan = []
    for j, (b, _l) in enumerate(jobs):
        if j == 1:
            dve += 2 * (2 * QW * 1.042 + 125.0)
            act += 2 * (2 * QW * 0.833 + 212.0)
        if j >= 2:
            act += pads_c[jobs[j - 2][0]] * 0.833 + 212.0
        row = []
        for _k in range(CH):
            cd = pads_c[b] * 1.042 + 125.0
            ca = pads_c[b] * 0.833 + 212.0
            if dve + cd <= act + ca:
                dve += cd
                row.append("dve")
            else:
                act += ca
                row.append("act")
        plan.append(row)
    return plan


def _build(pads_c: tuple, qpad: int = A, reps: int = 1):
    """Per-core SPMD program. pads_c[b]: compute width (mult of 16) of batch
    slot b. reps > 1 re-emits the whole body (timing instrumentation only)."""
    pads_c = tuple(int(p) for p in pads_c)
    pad_cmax = max(pads_c)
    slot_splits = [_col_splits(p) for p in pads_c]
    QW = BPC * L * qpad
    sign_plan = _sign_plan(pads_c, qpad)

    nc = bacc.Bacc("TRN2", target_bir_lowering=False, debug=False)

    QE = nc.dram_tensor("qe", [D, QW], F32R, kind="ExternalInput").ap()
    DE = nc.dram_tensor("de", [BPC, L, D, pad_cmax], F32R, kind="ExternalInput").ap()
    RT = nc.dram_tensor("rt", [D, NB], F32R, kind="ExternalInput").ap()
    OUT = nc.dram_tensor("out", [BPC, L, qpad, pad_cmax], BF16, kind="ExternalOutput").ap()

    def sign_to(eng, u, pp):
        if eng == "dve":
            nc.vector.tensor_scalar(u, pp, 1.0, -1.0, Alu.min, Alu.max)
        else:
            nc.scalar.activation(u, pp, Act.Sign)

    with tile.TileContext(nc) as tc, ExitStack() as ctx:
        const = ctx.enter_context(tc.tile_pool(name="const", bufs=1))
        ehp = ctx.enter_context(tc.tile_pool(name="ehp", bufs=4))
        u2p = ctx.enter_context(tc.tile_pool(name="u2p", bufs=3))
        outp = ctx.enter_context(tc.tile_pool(name="outp", bufs=2))
        # 8 PSUM banks: 2 x 2-bank rotating chunk tiles + 2 x 2-bank S tiles
        # (two S bufs so job j's dots never wait on job j-1's Sin drain)
        ps_p = ctx.enter_context(tc.tile_pool(name="ps_p", bufs=3, space="PSUM"))
        ps_s = ctx.enter_context(tc.tile_pool(name="ps_s", bufs=1, space="PSUM"))

        for _rep in range(reps):
            _rp = f"r{_rep}_"
            # ---- constants: rt arrives in pieces so the first projection
            # chunk unblocks as early as possible; everything lands directly
            # in float32r tiles (host pre-rounds to tf32) ----
            rhl = const.tile([D, NB], F32R, tag="rhl", name=f"{_rp}rhl")
            qh = const.tile([D, QW], F32R, tag="qh", name=f"{_rp}qh")
            U1 = const.tile([D, CH * QW], FP8, tag="U1", name=f"{_rp}U1")

            _slot_order = sorted(range(BPC), key=lambda s: -pads_c[s])
            jobs = [(b, l) for b in _slot_order for l in range(L)]
            st = [dict() for _ in jobs]

            def stage_a(j):
                b, l = jobs[j]
                pad = pads_c[b]
                eh = ehp.tile([D, pad_cmax], F32R, tag="eh",
                              name=f"{_rp}eh{j}")[:, 0:pad]
                nc.sync.dma_start(out=eh, in_=DE[b, l, :, 0:pad])
                st[j]["eh"] = eh

            # DMA priority order: first doc job, first proj chunk weights,
            # the rest of the weights, second doc job, queries.
            stage_a(0)
            nc.sync.dma_start(out=rhl[:, 0:128], in_=RT[:, 0:128])
            nc.sync.dma_start(out=rhl[:, 128:NB], in_=RT[:, 128:NB])
            nc.sync.dma_start(out=qh, in_=QE)
            stage_a(1)

            # PE pre-warm: dependency-free dummy matmuls pull the PE through
            # its cold/mid clock ramp while the first DMAs land, so the real
            # projections run at 2.4 GHz
            warm = const.tile([D, 512], BF16, tag="warm", name=f"{_rp}warm")
            nc.gpsimd.memset(warm, 0.0)
            # dummy Sign + Sin on the idle ACT engine so both act-table
            # loads are hoisted into the warm-up window instead of stalling
            # the pipeline at their first real use
            wact = const.tile([D, 32], BF16, tag="wact", name=f"{_rp}wact")
            nc.scalar.activation(wact[:, 0:16], warm[:, 0:16], Act.Sign)
            nc.scalar.activation(wact[:, 16:32], warm[:, 16:32], Act.Sin)
            wps = ps_p.tile([D, 1024], F32, tag="pp",
                            name=f"{_rp}wps")[:, 0:512]
            for i in range(NWARM):
                nc.tensor.matmul(wps, warm[:, 0:128], warm,
                                 start=True, stop=True)

            def stage_b(j):
                b, l = jobs[j]
                pad = pads_c[b]
                splits = slot_splits[b]
                npieces = len(splits)
                w = splits[0][1] - splits[0][0]
                assert npieces * w == pad
                eh = st[j]["eh"]
                U2 = u2p.tile([D, CH * pad_cmax], FP8, tag="U2",
                              name=f"{_rp}U2{j}")
                for k in range(CH):
                    rh_k = rhl[:, k * 128:(k + 1) * 128]
                    pp = ps_p.tile([D, 1024], F32, tag="pp",
                                   name=f"{_rp}pp{j}_{k}")
                    for c0, c1, p0 in splits:
                        nc.tensor.matmul(pp[:, p0:p0 + c1 - c0], rh_k,
                                         eh[:, c0:c1], start=True, stop=True)
                    if npieces == 1:
                        ppv = pp[:, 0:pad]
                        u2v = U2[:, k * pad:(k + 1) * pad]
                    else:
                        ppv = pp[:].rearrange("p (n c) -> p n c",
                                              c=512)[:, 0:npieces, 0:w]
                        u2v = U2[:, k * pad:(k + 1) * pad] \
                            .rearrange("p (n c) -> p n c", c=w)
                    sign_to(sign_plan[j][k], u2v, ppv)
                st[j]["U2"] = U2

            def query_proj():
                # chunk pairs share one PSUM tile (cols 0 and 512) so the
                # sign runs as one fused op per pair, alternating DVE/ACT
                # so all four signs finish before the first dot needs U1
                for kk in range(CH // 2):
                    qp = ps_p.tile([D, 1024], F32, tag="pp",
                                   name=f"{_rp}qp{kk}")
                    for h in range(2):
                        k = 2 * kk + h
                        nc.tensor.matmul(qp[:, 512 * h:512 * h + QW],
                                         rhl[:, k * 128:(k + 1) * 128], qh,
                                         start=True, stop=True)
                    u1v = U1[:, 2 * kk * QW:(2 * kk + 2) * QW] \
                        .rearrange("p (n c) -> p n c", c=QW)
                    qpv = qp[:].rearrange("p (n c) -> p n c",
                                          c=512)[:, 0:2, 0:QW]
                    sign_to("dve" if kk % 2 == 0 else "act", u1v, qpv)

            def stage_c(j):
                b, l = jobs[j]
                pad = pads_c[b]
                splits = slot_splits[b]
                npieces = len(splits)
                w = splits[0][1] - splits[0][0]
                U2 = st[j]["U2"]
                # code dot via fp8 DoubleRow: chunk pairs (2jj, 2jj+1) fold
                # into one K=256 matmul; +-1/0 codes are exact in fp8e4m3
                S = ps_s.tile([qpad, 1024], F32, tag="S",
                              name=f"{_rp}S{j}")
                qcol = (b * L + l) * qpad
                for c0, c1, p0 in splits:
                    for jj in range(CH // 2):
                        lw = U1[:, 2 * jj * QW:(2 * jj + 2) * QW] \
                            .rearrange("p (o c) -> p o c", o=2)[:, :, qcol:qcol + qpad]
                        rv = U2[:, 2 * jj * pad:(2 * jj + 2) * pad] \
                            .rearrange("p (o c) -> p o c", o=2)[:, :, c0:c1]
                        nc.tensor.matmul(
                            S[:, p0:p0 + c1 - c0], lw, rv,
                            start=(jj == 0), stop=(jj == CH // 2 - 1),
                            perf_mode=mybir.MatmulPerfMode.DoubleRow,
                        )
                sim = outp.tile([qpad, pad_cmax], BF16, tag="sim",
                                name=f"{_rp}sim{j}")[:, 0:pad]
                if npieces == 1:
                    nc.scalar.activation(sim, S[:, 0:pad], Act.Sin,
                                         scale=PI / (2.0 * NB))
                else:
                    sv = S[:].rearrange("p (n c) -> p n c",
                                        c=512)[:, 0:npieces, 0:w]
                    mv = sim.rearrange("p (n c) -> p n c", c=w)
                    nc.scalar.activation(mv, sv, Act.Sin, scale=PI / (2.0 * NB))
                nc.sync.dma_start(out=OUT[b, l, :, 0:pad], in_=sim)

            # deeper pipeline: c(j) trails b(j+1), so dots/sin/output of job
            # j overlap the projections of job j+2 and the PE never waits
            # on the sign engines at job boundaries
            n = len(jobs)
            stage_b(0)
            stage_a(2)
            query_proj()
            stage_b(1)
            stage_a(3)
            for j in range(n):
                stage_c(j)
                if j + 4 < n:
                    stage_a(j + 4)
                if j + 2 < n:
                    stage_b(j + 2)

    nc.compile()
    return nc


def _tf32(x):
    """Round-to-nearest-even fp32 -> tf32 (11-bit mantissa), bit-matching
    the PE's fp32_to_fp32r conversion."""
    u = np.ascontiguousarray(x, np.float32).view(np.uint32).astype(np.uint64)
    u = (u + 0x07FF + ((u >> 12) & 1)) & 0xFFFFFFFFFFFFF000
    return (u & 0xFFFFFFFF).astype(np.uint32).view(np.float32)


def _stage_inputs(query_embed, doc_embed, query_tok, doc_tok, r):
    query_embed = np.ascontiguousarray(query_embed, dtype=np.float32)
    doc_embed = np.ascontiguousarray(doc_embed, dtype=np.float32)
    r = np.ascontiguousarray(r, dtype=np.float32)

    qmask = (np.asarray(query_tok) != 0)
    dmask = (np.asarray(doc_tok) != 0)

    # sort batches by active count; slot s takes ranks [s*CORES, (s+1)*CORES)
    # spread across the 8 cores, so per-slot padding is tight and identical
    # on every core (SPMD requires one shape per slot)
    counts = dmask.sum(axis=1).astype(int)
    order = np.argsort(counts, kind="stable")
    assign = np.empty((CORES, BPC), dtype=int)   # assign[c, b] = batch id
    for s in range(BPC):
        for c in range(CORES):
            assign[c, s] = order[s * CORES + c]
    def _pad(n):
        # mult of 16; slots that split across two PSUM banks need halves
        # that are themselves mult of 16, so round those to mult of 32
        p = max(64, -(-n // 16) * 16)
        if p > 512:
            p = -(-n // 32) * 32
        return min(BDOC, p)

    pads_c = tuple(_pad(int(counts[assign[:, s]].max())) for s in range(BPC))
    pad_cmax = max(pads_c)

    qe_m = query_embed * qmask[None, :, :, None].astype(np.float32)
    qidxs = [np.flatnonzero(qmask[g]) for g in range(BAT)]
    qpad = min(A, max(16, int(-(-max(len(q) for q in qidxs) // 16) * 16)))
    rt = np.ascontiguousarray(_tf32(r.T * SCALE))

    idxs = [np.flatnonzero(dmask[g]) for g in range(BAT)]
    in_maps = []
    for c in range(CORES):
        # embeddings staged pre-transposed [D, tokens], pre-rounded to tf32
        # (value-exact under the f32r DMA interpretation); queries compacted
        # to their active rows (masks are per-batch, shared by both layers)
        qe_c = np.zeros((D, BPC * L * qpad), dtype=np.float32)
        de_c = np.zeros((BPC, L, D, pad_cmax), dtype=np.float32)
        for b in range(BPC):
            g = assign[c, b]
            qi = qidxs[g]
            for li in range(L):
                col = (b * L + li) * qpad
                qe_c[:, col:col + len(qi)] = qe_m[li, g, qi].T
            idx = idxs[g]
            de_c[b, :, :, :len(idx)] = doc_embed[:, g, idx].transpose(0, 2, 1)
        in_maps.append({"qe": _tf32(qe_c), "de": _tf32(de_c), "rt": rt})

    return in_maps, assign, idxs, pads_c, qidxs, qpad


def kernel(query_embed, doc_embed, query_tok, doc_tok, r):
    in_maps, assign, idxs, pads_c, qidxs, qpad = _stage_inputs(
        query_embed, doc_embed, query_tok, doc_tok, r)

    key = (pads_c, qpad)
    if key not in _BUILD_CACHE:
        _BUILD_CACHE[key] = _build(pads_c, qpad)
    nc = _BUILD_CACHE[key]

    res = run_bass_kernel_spmd(nc, in_maps, core_ids=list(range(CORES)))

    out = np.zeros((BAT, L, A, BDOC), dtype=np.float32)
    for c in range(CORES):
        o_c = np.asarray(res.results[c]["out"]).astype(np.float32)
        for b in range(BPC):
            g = assign[c, b]
            idx = idxs[g]
            qi = qidxs[g]
            for li in range(L):
                out[g, li][np.ix_(qi, idx)] = o_c[b, li, :len(qi), :len(idx)]
    return out


# revision 24
# speedup vs baseline: 1.2879x; 1.0331x over previous
"""LSH similarity-matrix kernel for Trainium2 (8 NeuronCores, data-parallel over batch).

Math: reference computes, per (l, b):
    c1 = (query_embed @ r.T > 0),  c2 = (doc_embed @ r.T > 0)   in {0,1}
    ham = s1 + s2 - 2*c1@c2.T ;  sim = cos(pi/NB * ham), masked where tok==0.
With +-1 codes U = 2c-1 and S = U1 @ U2.T:  ham = (NB - S)/2, so
    sim = sin(pi/(2*NB) * S).
Masks fold into the embeddings: a zeroed embedding row projects to 0,
clamp(0) = 0 gives a zero code row, so S = 0 and sin(0) = 0 — exactly the
masked output. Masked doc tokens (half of them: tok in {0,1}) are gathered
away host-side entirely; output columns scatter back as zeros. Batches are
assigned to (core, slot) sorted by active-token count so every core runs an
identically-shaped program with minimal padding per slot.

Precision: projections run as single tf32 (float32r) matmuls at 1 cycle/row.
tf32's 11-bit mantissa flips ~1.5k of the 71M hash bits vs the fp32
reference (sim absmax ~9e-3, rel err ~1e-4) — far inside the tolerance.
Inputs are pre-rounded to tf32 host-side and DMA'd straight into float32r
tiles, so no on-device conversion copies are needed. The code dot runs as
fp8e4m3 DoubleRow matmuls (chunk pairs give K=256 per MM at 2 MACs/cell/
cycle); +-1/0 codes and their fp32 PSUM accumulation are exact.

r is pre-scaled by 2^66 host-side so the sign alternative
clamp(x, -1, 1) = max(min(x,1),-1) is exact (any |proj| > 2^-66 maps to
+-1). The PSUM->SBUF sign drain is the second bottleneck after the PE
(GPSIMD cannot read PSUM, so only DVE and ACT can do it); chunk signs are
assigned to DVE (clamp tensor_scalar) or ACT (Sign activation) by a greedy
cost balancer that pre-charges ACT with the per-job Sin epilogue and its
act-table loads. Output is written bf16 (exact enough; halves output DMA)
and upcast on the host.
"""
import os
import sys

sys.path.insert(0, "/opt/trn_rl_repo")

from contextlib import ExitStack

import numpy as np

import concourse.bass as bass
import concourse.mybir as mybir
import concourse.tile as tile
from concourse import bacc
from concourse.bass_utils import run_bass_kernel_spmd

L, BAT, A, BDOC, D, NB = 2, 32, 64, 1024, 128, 1024
CORES = 8
BPC = BAT // CORES          # batch slots per core
CH = NB // 128              # 8 bit-chunks
SCALE = float(2.0 ** 66)
PI = float(np.pi)

F32 = mybir.dt.float32
F32R = mybir.dt.float32r
BF16 = mybir.dt.bfloat16
FP8 = mybir.dt.float8e4
Alu = mybir.AluOpType
Act = mybir.ActivationFunctionType

NWARM = 7                   # PE ramp dummy matmuls (512 cols each)

_BUILD_CACHE: dict = {}


def _col_splits(n):
    """Split [0, n) into equal-width pieces of <=512 columns (>=256 keeps
    float32r matmuls at full rate; a matmul may not cross a PSUM bank, so
    piece i is written at PSUM column 512*i). Equal widths mean one strided
    [p, npieces, w] access pattern covers all pieces, so sign/sin run as a
    single instruction per chunk. Returns (c0, c1, p0) per piece."""
    npieces = -(-n // 512)
    w = -(-(n // npieces) // 16) * 16
    while w * npieces < n:
        w += 16
    assert w * npieces >= n and w <= 512 and npieces <= 2
    return [(i * w, min((i + 1) * w, n), 512 * i) for i in range(npieces)]


def _sign_plan(pads_c, qpad):
    """Assign each (job, chunk) doc sign to 'dve' or 'act' greedily by
    modelled engine cost (ns): DVE tensor_scalar = free*1.042 + 125,
    ACT activation = free*0.833 + 143. Costs are charged at the point in
    the job stream where the work actually runs (sin of job j-1 lands
    during job j; query-pair signs land around job 1; act-table loads are
    hoisted to the idle warm-up window) so the split is balanced in TIME,
    not just in total. Jobs run slots in descending-pad order, L-major."""
    QW = BPC * L * qpad
    order = sorted(range(BPC), key=lambda s: -pads_c[s])
    jobs = [(b, l) for b in order for l in range(L)]
    dve = 0.0
    act = 0.0
    plan = []
    for j, (b, _l) in enumerate(jobs):
        if j == 2:
            dve += 2 * (2 * QW * 1.042 + 125.0)
            act += 2 * (2 * QW * 0.833 + 212.0)
        if j >= 2:
            act += pads_c[jobs[j - 2][0]] * 0.833 + 212.0
        row = []
        for _k in range(CH):
            cd = pads_c[b] * 1.042 + 125.0
            ca = pads_c[b] * 0.833 + 212.0
            if dve + cd <= act + ca:
                dve += cd
                row.append("dve")
            else:
                act += ca
                row.append("act")
        plan.append(row)
    return plan


def _build(pads_c: tuple, qpad: int = A, reps: int = 1):
    """Per-core SPMD program. pads_c[b]: compute width (mult of 16) of batch
    slot b. reps > 1 re-emits the whole body (timing instrumentation only)."""
    pads_c = tuple(int(p) for p in pads_c)
    pad_cmax = max(pads_c)
    slot_splits = [_col_splits(p) for p in pads_c]
    QW = BPC * L * qpad
    sign_plan = _sign_plan(pads_c, qpad)

    nc = bacc.Bacc("TRN2", target_bir_lowering=False, debug=False)

    QE = nc.dram_tensor("qe", [D, QW], F32R, kind="ExternalInput").ap()
    DE = nc.dram_tensor("de", [BPC, L, D, pad_cmax], F32R, kind="ExternalInput").ap()
    RT = nc.dram_tensor("rt", [D, NB], F32R, kind="ExternalInput").ap()
    OUT = nc.dram_tensor("out", [BPC, L, qpad, pad_cmax], BF16, kind="ExternalOutput").ap()

    def sign_to(eng, u, pp):
        if eng == "dve":
            nc.vector.tensor_scalar(u, pp, 1.0, -1.0, Alu.min, Alu.max)
        else:
            nc.scalar.activation(u, pp, Act.Sign)

    with tile.TileContext(nc) as tc, ExitStack() as ctx:
        const = ctx.enter_context(tc.tile_pool(name="const", bufs=1))
        ehp = ctx.enter_context(tc.tile_pool(name="ehp", bufs=4))
        u2p = ctx.enter_context(tc.tile_pool(name="u2p", bufs=3))
        outp = ctx.enter_context(tc.tile_pool(name="outp", bufs=2))
        # 8 PSUM banks: 2 x 2-bank rotating chunk tiles + 2 x 2-bank S tiles
        # (two S bufs so job j's dots never wait on job j-1's Sin drain)
        ps_p = ctx.enter_context(tc.tile_pool(name="ps_p", bufs=3, space="PSUM"))
        ps_s = ctx.enter_context(tc.tile_pool(name="ps_s", bufs=1, space="PSUM"))

        for _rep in range(reps):
            _rp = f"r{_rep}_"
            # ---- constants: rt arrives in pieces so the first projection
            # chunk unblocks as early as possible; everything lands directly
            # in float32r tiles (host pre-rounds to tf32) ----
            rhl = const.tile([D, NB], F32R, tag="rhl", name=f"{_rp}rhl")
            qh = const.tile([D, QW], F32R, tag="qh", name=f"{_rp}qh")
            U1 = const.tile([D, CH * QW], FP8, tag="U1", name=f"{_rp}U1")

            _slot_order = sorted(range(BPC), key=lambda s: -pads_c[s])
            jobs = [(b, l) for b in _slot_order for l in range(L)]
            st = [dict() for _ in jobs]

            def stage_a(j):
                b, l = jobs[j]
                pad = pads_c[b]
                eh = ehp.tile([D, pad_cmax], F32R, tag="eh",
                              name=f"{_rp}eh{j}")[:, 0:pad]
                nc.sync.dma_start(out=eh, in_=DE[b, l, :, 0:pad])
                st[j]["eh"] = eh

            # DMA priority order: first doc job, first proj chunk weights,
            # the rest of the weights, second doc job, queries.
            nc.sync.dma_start(out=rhl[:, 0:128], in_=RT[:, 0:128])
            stage_a(0)
            nc.sync.dma_start(out=rhl[:, 128:512], in_=RT[:, 128:512])
            nc.sync.dma_start(out=rhl[:, 512:NB], in_=RT[:, 512:NB])
            stage_a(1)
            nc.sync.dma_start(out=qh, in_=QE)

            # PE pre-warm: dependency-free dummy matmuls pull the PE through
            # its cold/mid clock ramp while the first DMAs land, so the real
            # projections run at 2.4 GHz
            warm = const.tile([D, 512], BF16, tag="warm", name=f"{_rp}warm")
            nc.gpsimd.memset(warm, 0.0)
            # dummy Sign + Sin on the idle ACT engine so both act-table
            # loads are hoisted into the warm-up window instead of stalling
            # the pipeline at their first real use
            wact = const.tile([D, 32], BF16, tag="wact", name=f"{_rp}wact")
            nc.scalar.activation(wact[:, 0:16], warm[:, 0:16], Act.Sign)
            nc.scalar.activation(wact[:, 16:32], warm[:, 16:32], Act.Sin)
            wps = ps_p.tile([D, 1024], F32, tag="pp",
                            name=f"{_rp}wps")[:, 0:512]
            for i in range(NWARM):
                nc.tensor.matmul(wps, warm[:, 0:128], warm,
                                 start=True, stop=True)

            def stage_b(j, ks):
                b, l = jobs[j]
                pad = pads_c[b]
                splits = slot_splits[b]
                npieces = len(splits)
                w = splits[0][1] - splits[0][0]
                assert npieces * w == pad
                eh = st[j]["eh"]
                if "U2" not in st[j]:
                    st[j]["U2"] = u2p.tile([D, CH * pad_cmax], FP8, tag="U2",
                                           name=f"{_rp}U2{j}")
                U2 = st[j]["U2"]
                for k in ks:
                    rh_k = rhl[:, k * 128:(k + 1) * 128]
                    pp = ps_p.tile([D, 1024], F32, tag="pp",
                                   name=f"{_rp}pp{j}_{k}")
                    for c0, c1, p0 in splits:
                        nc.tensor.matmul(pp[:, p0:p0 + c1 - c0], rh_k,
                                         eh[:, c0:c1], start=True, stop=True)
                    if npieces == 1:
                        ppv = pp[:, 0:pad]
                        u2v = U2[:, k * pad:(k + 1) * pad]
                    else:
                        ppv = pp[:].rearrange("p (n c) -> p n c",
                                              c=512)[:, 0:npieces, 0:w]
                        u2v = U2[:, k * pad:(k + 1) * pad] \
                            .rearrange("p (n c) -> p n c", c=w)
                    sign_to(sign_plan[j][k], u2v, ppv)

            def query_proj():
                # chunk pairs share one PSUM tile (cols 0 and 512) so the
                # sign runs as one fused op per pair, alternating DVE/ACT
                # so all four signs finish before the first dot needs U1
                for kk in range(CH // 2):
                    qp = ps_p.tile([D, 1024], F32, tag="pp",
                                   name=f"{_rp}qp{kk}")
                    for h in range(2):
                        k = 2 * kk + h
                        nc.tensor.matmul(qp[:, 512 * h:512 * h + QW],
                                         rhl[:, k * 128:(k + 1) * 128], qh,
                                         start=True, stop=True)
                    u1v = U1[:, 2 * kk * QW:(2 * kk + 2) * QW] \
                        .rearrange("p (n c) -> p n c", c=QW)
                    qpv = qp[:].rearrange("p (n c) -> p n c",
                                          c=512)[:, 0:2, 0:QW]
                    sign_to("dve" if kk % 2 == 0 else "act", u1v, qpv)

            def stage_c(j):
                b, l = jobs[j]
                pad = pads_c[b]
                splits = slot_splits[b]
                npieces = len(splits)
                w = splits[0][1] - splits[0][0]
                U2 = st[j]["U2"]
                # code dot via fp8 DoubleRow: chunk pairs (2jj, 2jj+1) fold
                # into one K=256 matmul; +-1/0 codes are exact in fp8e4m3
                S = ps_s.tile([qpad, 1024], F32, tag="S",
                              name=f"{_rp}S{j}")
                qcol = (b * L + l) * qpad
                for c0, c1, p0 in splits:
                    for jj in range(CH // 2):
                        lw = U1[:, 2 * jj * QW:(2 * jj + 2) * QW] \
                            .rearrange("p (o c) -> p o c", o=2)[:, :, qcol:qcol + qpad]
                        rv = U2[:, 2 * jj * pad:(2 * jj + 2) * pad] \
                            .rearrange("p (o c) -> p o c", o=2)[:, :, c0:c1]
                        nc.tensor.matmul(
                            S[:, p0:p0 + c1 - c0], lw, rv,
                            start=(jj == 0), stop=(jj == CH // 2 - 1),
                            perf_mode=mybir.MatmulPerfMode.DoubleRow,
                        )
                sim = outp.tile([qpad, pad_cmax], BF16, tag="sim",
                                name=f"{_rp}sim{j}")[:, 0:pad]
                if npieces == 1:
                    nc.scalar.activation(sim, S[:, 0:pad], Act.Sin,
                                         scale=PI / (2.0 * NB))
                else:
                    sv = S[:].rearrange("p (n c) -> p n c",
                                        c=512)[:, 0:npieces, 0:w]
                    mv = sim.rearrange("p (n c) -> p n c", c=w)
                    nc.scalar.activation(mv, sv, Act.Sin, scale=PI / (2.0 * NB))
                nc.sync.dma_start(out=OUT[b, l, :, 0:pad], in_=sim)

            # deeper pipeline: c(j) is emitted in the MIDDLE of b(j+2), so
            # job j's dots run while job j+2's early signs drain and the Sin
            # epilogue lands mid-stream on ACT instead of stalling it
            n = len(jobs)
            LO, HI = range(0, CH // 2), range(CH // 2, CH)
            stage_b(0, range(CH))
            stage_a(2)
            stage_b(1, range(CH))
            query_proj()
            stage_a(3)
            for j in range(n):
                if j + 2 < n:
                    stage_b(j + 2, LO)
                stage_c(j)
                if j + 4 < n:
                    stage_a(j + 4)
                if j + 2 < n:
                    stage_b(j + 2, HI)

    nc.compile()
    return nc


def _tf32(x):
    """Round-to-nearest-even fp32 -> tf32 (11-bit mantissa), bit-matching
    the PE's fp32_to_fp32r conversion."""
    u = np.ascontiguousarray(x, np.float32).view(np.uint32).astype(np.uint64)
    u = (u + 0x07FF + ((u >> 12) & 1)) & 0xFFFFFFFFFFFFF000
    return (u & 0xFFFFFFFF).astype(np.uint32).view(np.float32)


def _stage_inputs(query_embed, doc_embed, query_tok, doc_tok, r):
    query_embed = np.ascontiguousarray(query_embed, dtype=np.float32)
    doc_embed = np.ascontiguousarray(doc_embed, dtype=np.float32)
    r = np.ascontiguousarray(r, dtype=np.float32)

    qmask = (np.asarray(query_tok) != 0)
    dmask = (np.asarray(doc_tok) != 0)

    # sort batches by active count; slot s takes ranks [s*CORES, (s+1)*CORES)
    # spread across the 8 cores, so per-slot padding is tight and identical
    # on every core (SPMD requires one shape per slot)
    counts = dmask.sum(axis=1).astype(int)
    order = np.argsort(counts, kind="stable")
    assign = np.empty((CORES, BPC), dtype=int)   # assign[c, b] = batch id
    for s in range(BPC):
        for c in range(CORES):
            assign[c, s] = order[s * CORES + c]
    def _pad(n):
        # mult of 16; slots that split across two PSUM banks need halves
        # that are themselves mult of 16, so round those to mult of 32
        p = max(64, -(-n // 16) * 16)
        if p > 512:
            p = -(-n // 32) * 32
        return min(BDOC, p)

    pads_c = tuple(_pad(int(counts[assign[:, s]].max())) for s in range(BPC))
    pad_cmax = max(pads_c)

    qe_m = query_embed * qmask[None, :, :, None].astype(np.float32)
    qidxs = [np.flatnonzero(qmask[g]) for g in range(BAT)]
    qpad = min(A, max(16, int(-(-max(len(q) for q in qidxs) // 16) * 16)))
    rt = np.ascontiguousarray(_tf32(r.T * SCALE))

    idxs = [np.flatnonzero(dmask[g]) for g in range(BAT)]
    in_maps = []
    for c in range(CORES):
        # embeddings staged pre-transposed [D, tokens], pre-rounded to tf32
        # (value-exact under the f32r DMA interpretation); queries compacted
        # to their active rows (masks are per-batch, shared by both layers)
        qe_c = np.zeros((D, BPC * L * qpad), dtype=np.float32)
        de_c = np.zeros((BPC, L, D, pad_cmax), dtype=np.float32)
        for b in range(BPC):
            g = assign[c, b]
            qi = qidxs[g]
            for li in range(L):
                col = (b * L + li) * qpad
                qe_c[:, col:col + len(qi)] = qe_m[li, g, qi].T
            idx = idxs[g]
            de_c[b, :, :, :len(idx)] = doc_embed[:, g, idx].transpose(0, 2, 1)
        in_maps.append({"qe": _tf32(qe_c), "de": _tf32(de_c), "rt": rt})

    return in_maps, assign, idxs, pads_c, qidxs, qpad


def kernel(query_embed, doc_embed, query_tok, doc_tok, r):
    in_maps, assign, idxs, pads_c, qidxs, qpad = _stage_inputs(
        query_embed, doc_embed, query_tok, doc_tok, r)

    key = (pads_c, qpad)
    if key not in _BUILD_CACHE:
        _BUILD_CACHE[key] = _build(pads_c, qpad)
    nc = _BUILD_CACHE[key]

    res = run_bass_kernel_spmd(nc, in_maps, core_ids=list(range(CORES)))

    out = np.zeros((BAT, L, A, BDOC), dtype=np.float32)
    for c in range(CORES):
        o_c = np.asarray(res.results[c]["out"]).astype(np.float32)
        for b in range(BPC):
            g = assign[c, b]
            idx = idxs[g]
            qi = qidxs[g]
            for li in range(L):
                out[g, li][np.ix_(qi, idx)] = o_c[b, li, :len(qi), :len(idx)]
    return out
